# revision 4
# baseline (speedup 1.0000x reference)
"""HRA-injected linear on 8 Trainium2 NeuronCores.

Math: reference applies r=8 sequential Householder updates to W, then y = x @ W'^T.
Compact WY form (exact): W' = W (I - V U^T), so W'^T = W^T - U (V^T W^T), with
U = normalized hra_u columns and V from a tiny host-side recursion.

Per core (8 cores = 4 token-groups x 2 out-feature-groups):
  y_s [2048, 2048] f32 = x_s [2048, 4096] @ W'_s^T

Device program (all transposes via DMA xbar, PE does only matmuls):
  for each 512-wide output block (4 passes):
    wt   <- xbar-transpose of W rows            [128, 32k, 512] bf16 (SBUF)
    wvn  <- (-V)^T @ wt   (32 matmuls)          [8, 512]
    wt   += U @ wvn       (32 K=8 matmuls + DVE adds)   == W'^T block
    for each of 16 token tiles:
      xt  <- xbar-transpose of x rows           [128, 32k, 128] bf16
      psum = sum_k xt[k].T @ wt[k]  (32 matmuls, PSUM accumulate)
      y block <- psum
"""

import numpy as np
import ml_dtypes
from contextlib import ExitStack

import concourse.bacc as bacc
import concourse.mybir as mybir
import concourse.tile as tile
from concourse.bass_utils import run_bass_kernel_spmd

P = 128
D = 4096          # in_features (contraction)
R = 8             # Householder rank
TOK = 8192        # 4*2048 flattened tokens
O = 4096          # out_features
TOK_GROUPS = 4
O_GROUPS = 2
TOK_S = TOK // TOK_GROUPS   # 2048 tokens per core
O_S = O // O_GROUPS         # 2048 out features per core
KT = D // P                 # 32 contraction tiles
MT = TOK_S // P             # 16 token tiles per core
NBW = 512                   # output block width
NB = O_S // NBW             # 4 passes

F32 = mybir.dt.float32
BF16 = mybir.dt.bfloat16

N_CORES = 8

_NC = None


def _build():
    nc = bacc.Bacc(None, target_bir_lowering=False)
    x_d = nc.declare_dram_parameter("x", [TOK_S, D], BF16, isOutput=False)
    w_d = nc.declare_dram_parameter("w", [O_S, D], BF16, isOutput=False)
    ut_d = nc.declare_dram_parameter("ut", [R, D], BF16, isOutput=False)
    vn_d = nc.declare_dram_parameter("vn", [P, KT, R], BF16, isOutput=False)
    y_d = nc.declare_dram_parameter("out", [TOK_S, O_S], F32, isOutput=True)

    with tile.TileContext(nc) as tc, ExitStack() as ctx:
        const = ctx.enter_context(tc.tile_pool(name="const", bufs=1))
        wt_pool = ctx.enter_context(tc.tile_pool(name="wtp", bufs=2))
        xt_pool = ctx.enter_context(tc.tile_pool(name="xtp", bufs=3))
        wvp = ctx.enter_context(tc.tile_pool(name="wvp", bufs=2))
        ysb = ctx.enter_context(tc.tile_pool(name="ysb", bufs=4))
        psum = ctx.enter_context(tc.tile_pool(name="psum", bufs=1, space="PSUM"))

        ut_sb = const.tile([R, D], BF16)
        nc.sync.dma_start(out=ut_sb, in_=ut_d[:])
        vn_sb = const.tile([P, KT, R], BF16)
        nc.sync.dma_start(out=vn_sb, in_=vn_d[:])

        for nb in range(NB):
            # W^T block via xbar: wt[p, k, t] = W[nb*512 + t, k*128 + p]
            wt = wt_pool.tile([P, KT, NBW], BF16, tag="wt")
            nc.sync.dma_start(
                out=wt, in_=w_d[nb * NBW:(nb + 1) * NBW, :], transpose=True
            )

            # wvn = (-V)^T @ W^T block  [8, 512]
            ps_wv = psum.tile([R, NBW], F32, tag="pwv", bufs=2, name="ps_wv")
            for k in range(KT):
                nc.tensor.matmul(ps_wv, vn_sb[:, k, :], wt[:, k, :],
                                 start=(k == 0), stop=(k == KT - 1))
            wvn = wvp.tile([R, NBW], BF16, tag="wvn")
            nc.vector.tensor_copy(out=wvn, in_=ps_wv)

            # fold the rank-8 update: wt += U-chunk @ wvn  (per k)
            for k in range(KT):
                ps_f = psum.tile([P, NBW], F32, tag="pf", bufs=2, name="ps_f")
                nc.tensor.matmul(ps_f, ut_sb[:, k * P:(k + 1) * P], wvn,
                                 start=True, stop=True)
                nc.vector.tensor_tensor(
                    out=wt[:, k, :], in0=wt[:, k, :], in1=ps_f,
                    op=mybir.AluOpType.add,
                )

            for m in range(MT):
                xt = xt_pool.tile([P, KT, P], BF16, tag="xt")
                nc.sync.dma_start(out=xt, in_=x_d[m * P:(m + 1) * P, :],
                                  transpose=True)
                ps_y = psum.tile([P, NBW], F32, tag="py", bufs=4, name="ps_y")
                for k in range(KT):
                    nc.tensor.matmul(ps_y, xt[:, k, :], wt[:, k, :],
                                     start=(k == 0), stop=(k == KT - 1))
                y_t = ysb.tile([P, NBW], F32, tag="y")
                nc.vector.tensor_copy(out=y_t, in_=ps_y)
                nc.sync.dma_start(
                    out=y_d[m * P:(m + 1) * P, nb * NBW:(nb + 1) * NBW], in_=y_t
                )

    nc.compile()
    return nc


def _get_nc():
    global _NC
    if _NC is None:
        _NC = _build()
    return _NC


def _host_prep(hra_u):
    """Normalize u columns and compute V of the compact WY form, in float64."""
    u = hra_u.astype(np.float64)
    u = u / np.linalg.norm(u, axis=0, keepdims=True)        # [D, R]
    v = np.zeros_like(u)
    for k_ in range(R):
        acc = u[:, k_].copy()
        for j in range(k_):
            acc -= v[:, j] * np.dot(u[:, j], u[:, k_])
        v[:, k_] = 2.0 * acc
    ut = np.ascontiguousarray(u.T).astype(ml_dtypes.bfloat16)          # [R, D]
    vn = np.ascontiguousarray(
        (-v).reshape(KT, P, R).transpose(1, 0, 2)
    ).astype(ml_dtypes.bfloat16)                                       # [P, KT, R]
    return ut, vn


def _make_in_maps(x, weight, hra_u):
    ut, vn = _host_prep(hra_u)
    xf = np.ascontiguousarray(x.reshape(TOK, D)).astype(ml_dtypes.bfloat16)
    wf = np.ascontiguousarray(weight).astype(ml_dtypes.bfloat16)

    in_maps = []
    for core in range(N_CORES):
        a, b = core // O_GROUPS, core % O_GROUPS
        in_maps.append({
            "x": np.ascontiguousarray(xf[a * TOK_S:(a + 1) * TOK_S]),
            "w": np.ascontiguousarray(wf[b * O_S:(b + 1) * O_S]),
            "ut": ut,
            "vn": vn,
        })
    return in_maps


def kernel(x, weight, hra_u):
    nc = _get_nc()
    in_maps = _make_in_maps(x, weight, hra_u)
    res = run_bass_kernel_spmd(nc, in_maps, core_ids=list(range(N_CORES))).results

    y = np.empty((TOK, O), dtype=np.float32)
    for core in range(N_CORES):
        a, b = core // O_GROUPS, core % O_GROUPS
        y[a * TOK_S:(a + 1) * TOK_S, b * O_S:(b + 1) * O_S] = res[core]["out"]
    return y.reshape(x.shape[0], x.shape[1], O)


# revision 5
# speedup vs baseline: 1.1991x; 1.1991x over previous
"""HRA-injected linear on 8 Trainium2 NeuronCores.

Math: reference applies r=8 sequential Householder updates to W, then y = x @ W'^T.
Compact WY form (exact): W' = W (I - V U^T), so W'^T = W^T - U (V^T W^T), with
U = normalized hra_u columns and V from a tiny host-side recursion.

Per core (8 cores = 4 token-groups x 2 out-feature-groups):
  y_s [2048, 2048] f32 = x_s [2048, 4096] @ W'_s^T

Device program (PE does only matmuls; transposes ride the DMA xbar):
  build W'^T resident in SBUF [128, 32k, 2048] bf16, per 512-wide block:
    load W rows -> SBUF stage -> xbar transpose -> wt
    wvn = (-V)^T @ wt block (32 matmuls)
    wt block += U @ wvn     (32 K=8 matmuls + DVE adds)  == W'^T
  for each of 16 token tiles:
    x rows -> stage -> xbar -> xt [128, 32k, 128]
    4 PSUM banks accumulate over k: y[m, nb] = sum_k xt[k].T @ wt[k, nb]
    evict via DVE, one 1MB DMA out per tile
"""

import numpy as np
import ml_dtypes
from contextlib import ExitStack

import concourse.bacc as bacc
import concourse.mybir as mybir
import concourse.tile as tile
from concourse.bass_utils import run_bass_kernel_spmd

P = 128
D = 4096          # in_features (contraction)
R = 8             # Householder rank
TOK = 8192        # 4*2048 flattened tokens
O = 4096          # out_features
TOK_GROUPS = 4
O_GROUPS = 2
TOK_S = TOK // TOK_GROUPS   # 2048 tokens per core
O_S = O // O_GROUPS         # 2048 out features per core
KT = D // P                 # 32 contraction tiles
MT = TOK_S // P             # 16 token tiles per core
NBW = 512                   # output block width
NB = O_S // NBW             # 4 blocks
OC = O_S // P               # 16 weight row chunks

F32 = mybir.dt.float32
BF16 = mybir.dt.bfloat16

N_CORES = 8

_NC = None


def _build():
    nc = bacc.Bacc(None, target_bir_lowering=False)
    x_d = nc.declare_dram_parameter("x", [TOK_S, D], BF16, isOutput=False)
    w_d = nc.declare_dram_parameter("w", [O_S, D], BF16, isOutput=False)
    ut_d = nc.declare_dram_parameter("ut", [R, D], BF16, isOutput=False)
    vn_d = nc.declare_dram_parameter("vn", [P, KT, R], BF16, isOutput=False)
    y_d = nc.declare_dram_parameter("out", [TOK_S, O_S], F32, isOutput=True)

    with tile.TileContext(nc) as tc, ExitStack() as ctx:
        const = ctx.enter_context(tc.tile_pool(name="const", bufs=1))
        wt_pool = ctx.enter_context(tc.tile_pool(name="wtp", bufs=1))
        stg = ctx.enter_context(tc.tile_pool(name="stg", bufs=3))
        xt_pool = ctx.enter_context(tc.tile_pool(name="xtp", bufs=3))
        wvp = ctx.enter_context(tc.tile_pool(name="wvp", bufs=2))
        ysb = ctx.enter_context(tc.tile_pool(name="ysb", bufs=2))
        psum = ctx.enter_context(tc.tile_pool(name="psum", bufs=1, space="PSUM"))

        ut_sb = const.tile([R, D], BF16)
        nc.scalar.dma_start(out=ut_sb, in_=ut_d[:])
        vn_sb = const.tile([P, KT, R], BF16)
        nc.scalar.dma_start(out=vn_sb, in_=vn_d[:])

        # resident W'^T: wt[p, k, o] = W'[o, k*128+p]
        wt = wt_pool.tile([P, KT, O_S], BF16)

        for nb in range(NB):
            for c in range(4):       # four 128-row chunks of this 512 block
                cc = nb * 4 + c
                wst = stg.tile([P, D], BF16, tag="st", name="wst")
                nc.scalar.dma_start(
                    out=wst, in_=w_d[cc * P:(cc + 1) * P, :]
                )
                nc.sync.dma_start(
                    out=wt[:, :, cc * P:(cc + 1) * P], in_=wst, transpose=True
                )

            # wvn = (-V)^T @ W^T block  [8, 512]
            ps_wv = psum.tile([R, NBW], F32, tag="pwv", bufs=2, name="ps_wv")
            for k in range(KT):
                nc.tensor.matmul(ps_wv, vn_sb[:, k, :],
                                 wt[:, k, nb * NBW:(nb + 1) * NBW],
                                 start=(k == 0), stop=(k == KT - 1))
            wvn = wvp.tile([R, NBW], BF16, tag="wvn")
            nc.vector.tensor_copy(out=wvn, in_=ps_wv)

            # fold the rank-8 update: wt[:, k, block] += U-chunk @ wvn
            for k in range(KT):
                ps_f = psum.tile([P, NBW], F32, tag="pf", bufs=2, name="ps_f")
                nc.tensor.matmul(ps_f, ut_sb[:, k * P:(k + 1) * P], wvn,
                                 start=True, stop=True)
                nc.vector.tensor_tensor(
                    wt[:, k, nb * NBW:(nb + 1) * NBW],
                    wt[:, k, nb * NBW:(nb + 1) * NBW],
                    ps_f,
                    mybir.AluOpType.add,
                )

        for m in range(MT):
            xst = stg.tile([P, D], BF16, tag="st", name="xst")
            nc.scalar.dma_start(out=xst, in_=x_d[m * P:(m + 1) * P, :])
            xt = xt_pool.tile([P, KT, P], BF16, tag="xt")
            nc.sync.dma_start(out=xt, in_=xst, transpose=True)

            ps_y = [
                psum.tile([P, NBW], F32, tag=f"py{nb}", bufs=1, name=f"ps_y{nb}")
                for nb in range(NB)
            ]
            for nb in range(NB):     # nb-major so m=0 chases the W build
                for k in range(KT):
                    nc.tensor.matmul(ps_y[nb], xt[:, k, :],
                                     wt[:, k, nb * NBW:(nb + 1) * NBW],
                                     start=(k == 0), stop=(k == KT - 1))
            y_t = ysb.tile([P, O_S], F32, tag="y")
            for nb in range(NB):
                nc.vector.tensor_copy(
                    out=y_t[:, nb * NBW:(nb + 1) * NBW], in_=ps_y[nb]
                )
            nc.scalar.dma_start(out=y_d[m * P:(m + 1) * P, :], in_=y_t)

    nc.compile()
    return nc


def _get_nc():
    global _NC
    if _NC is None:
        _NC = _build()
    return _NC


def _host_prep(hra_u):
    """Normalize u columns and compute V of the compact WY form, in float64."""
    u = hra_u.astype(np.float64)
    u = u / np.linalg.norm(u, axis=0, keepdims=True)        # [D, R]
    v = np.zeros_like(u)
    for k_ in range(R):
        acc = u[:, k_].copy()
        for j in range(k_):
            acc -= v[:, j] * np.dot(u[:, j], u[:, k_])
        v[:, k_] = 2.0 * acc
    ut = np.ascontiguousarray(u.T).astype(ml_dtypes.bfloat16)          # [R, D]
    vn = np.ascontiguousarray(
        (-v).reshape(KT, P, R).transpose(1, 0, 2)
    ).astype(ml_dtypes.bfloat16)                                       # [P, KT, R]
    return ut, vn


def _make_in_maps(x, weight, hra_u):
    ut, vn = _host_prep(hra_u)
    xf = np.ascontiguousarray(x.reshape(TOK, D)).astype(ml_dtypes.bfloat16)
    wf = np.ascontiguousarray(weight).astype(ml_dtypes.bfloat16)

    in_maps = []
    for core in range(N_CORES):
        a, b = core // O_GROUPS, core % O_GROUPS
        in_maps.append({
            "x": np.ascontiguousarray(xf[a * TOK_S:(a + 1) * TOK_S]),
            "w": np.ascontiguousarray(wf[b * O_S:(b + 1) * O_S]),
            "ut": ut,
            "vn": vn,
        })
    return in_maps


def kernel(x, weight, hra_u):
    nc = _get_nc()
    in_maps = _make_in_maps(x, weight, hra_u)
    res = run_bass_kernel_spmd(nc, in_maps, core_ids=list(range(N_CORES))).results

    y = np.empty((TOK, O), dtype=np.float32)
    for core in range(N_CORES):
        a, b = core // O_GROUPS, core % O_GROUPS
        y[a * TOK_S:(a + 1) * TOK_S, b * O_S:(b + 1) * O_S] = res[core]["out"]
    return y.reshape(x.shape[0], x.shape[1], O)


# revision 6
# speedup vs baseline: 1.4236x; 1.1872x over previous
"""HRA-injected linear on 8 Trainium2 NeuronCores.

Math: reference applies r=8 sequential Householder updates to W, then y = x @ W'^T.
Compact WY form (exact): W' = W (I - V U^T), so W'^T = W^T - U (V^T W^T), with
U = normalized hra_u columns and V from a tiny host-side recursion.

Per core (8 cores = 4 token-groups x 2 out-feature-groups):
  y_s [2048, 2048] f32 = x_s [2048, 4096] @ W'_s^T

Host marshals per-core shards: x bf16, W^T bf16 in the device tile layout
[P, NB, KT, NBW] (wt[p, nb, k, t] = W[nb*512 + t, k*128 + p]), U^T and -V bf16.

Device program (PE does only matmuls):
  per 512-wide output block: DMA W^T block; wvn = (-V)^T @ wt (32 MMs);
    fold rank-8 update in place: wt += U @ wvn (32 K=8 MMs + DVE adds) == W'^T
  per token tile (16): x rows -> SBUF -> xbar transpose -> xt [128, 32k, 128];
    4 PSUM banks accumulate over k; DVE evict; one 1MB DMA out.
"""

import numpy as np
import ml_dtypes
from contextlib import ExitStack

import concourse.bacc as bacc
import concourse.mybir as mybir
import concourse.tile as tile
from concourse.bass_utils import run_bass_kernel_spmd

P = 128
D = 4096          # in_features (contraction)
R = 8             # Householder rank
TOK = 8192        # 4*2048 flattened tokens
O = 4096          # out_features
TOK_GROUPS = 4
O_GROUPS = 2
TOK_S = TOK // TOK_GROUPS   # 2048 tokens per core
O_S = O // O_GROUPS         # 2048 out features per core
KT = D // P                 # 32 contraction tiles
MT = TOK_S // P             # 16 token tiles per core
NBW = 512                   # output block width
NB = O_S // NBW             # 4 blocks

F32 = mybir.dt.float32
BF16 = mybir.dt.bfloat16

N_CORES = 8

_NC = None


def _build():
    nc = bacc.Bacc(None, target_bir_lowering=False)
    x_d = nc.declare_dram_parameter("x", [TOK_S, D], BF16, isOutput=False)
    wt_d = nc.declare_dram_parameter("wt", [P, NB, KT, NBW], BF16, isOutput=False)
    ut_d = nc.declare_dram_parameter("ut", [R, D], BF16, isOutput=False)
    vn_d = nc.declare_dram_parameter("vn", [P, KT, R], BF16, isOutput=False)
    y_d = nc.declare_dram_parameter("out", [TOK_S, O_S], F32, isOutput=True)

    with tile.TileContext(nc) as tc, ExitStack() as ctx:
        const = ctx.enter_context(tc.tile_pool(name="const", bufs=1))
        wt_pool = ctx.enter_context(tc.tile_pool(name="wtp", bufs=1))
        stg = ctx.enter_context(tc.tile_pool(name="stg", bufs=3))
        xt_pool = ctx.enter_context(tc.tile_pool(name="xtp", bufs=3))
        wvp = ctx.enter_context(tc.tile_pool(name="wvp", bufs=2))
        ysb = ctx.enter_context(tc.tile_pool(name="ysb", bufs=2))
        psum = ctx.enter_context(tc.tile_pool(name="psum", bufs=1, space="PSUM"))

        ut_sb = const.tile([R, D], BF16)
        nc.scalar.dma_start(out=ut_sb, in_=ut_d[:])
        vn_sb = const.tile([P, KT, R], BF16)
        nc.scalar.dma_start(out=vn_sb, in_=vn_d[:])

        # resident W'^T: wt[p, nb, k, t] = W'[nb*512 + t, k*128 + p]
        wt = wt_pool.tile([P, NB, KT, NBW], BF16)

        for nb in range(NB):
            nc.scalar.dma_start(out=wt[:, nb], in_=wt_d[:, nb])

            # wvn = (-V)^T @ W^T block  [8, 512]
            ps_wv = psum.tile([R, NBW], F32, tag="pwv", bufs=2, name="ps_wv")
            for k in range(KT):
                nc.tensor.matmul(ps_wv, vn_sb[:, k, :], wt[:, nb, k, :],
                                 start=(k == 0), stop=(k == KT - 1))
            wvn = wvp.tile([R, NBW], BF16, tag="wvn")
            nc.vector.tensor_copy(out=wvn, in_=ps_wv)

            # fold the rank-8 update: wt[:, nb, k, :] += U-chunk @ wvn
            for k in range(KT):
                ps_f = psum.tile([P, NBW], F32, tag="pf", bufs=2, name="ps_f")
                nc.tensor.matmul(ps_f, ut_sb[:, k * P:(k + 1) * P], wvn,
                                 start=True, stop=True)
                nc.vector.tensor_tensor(
                    wt[:, nb, k, :], wt[:, nb, k, :], ps_f, mybir.AluOpType.add,
                )

        for m in range(MT):
            xst = stg.tile([P, D], BF16, tag="st", name="xst")
            nc.scalar.dma_start(out=xst, in_=x_d[m * P:(m + 1) * P, :])
            xt = xt_pool.tile([P, KT, P], BF16, tag="xt")
            nc.sync.dma_start(out=xt, in_=xst, transpose=True)

            ps_y = [
                psum.tile([P, NBW], F32, tag=f"py{nb}", bufs=1, name=f"ps_y{nb}")
                for nb in range(NB)
            ]
            for nb in range(NB):     # nb-major so m=0 chases the W build
                for k in range(KT):
                    nc.tensor.matmul(ps_y[nb], xt[:, k, :], wt[:, nb, k, :],
                                     start=(k == 0), stop=(k == KT - 1))
            y_t = ysb.tile([P, O_S], F32, tag="y")
            for nb in range(NB):
                nc.vector.tensor_copy(
                    out=y_t[:, nb * NBW:(nb + 1) * NBW], in_=ps_y[nb]
                )
            nc.scalar.dma_start(out=y_d[m * P:(m + 1) * P, :], in_=y_t)

    nc.compile()
    return nc


def _get_nc():
    global _NC
    if _NC is None:
        _NC = _build()
    return _NC


def _host_prep(hra_u):
    """Normalize u columns and compute V of the compact WY form, in float64."""
    u = hra_u.astype(np.float64)
    u = u / np.linalg.norm(u, axis=0, keepdims=True)        # [D, R]
    v = np.zeros_like(u)
    for k_ in range(R):
        acc = u[:, k_].copy()
        for j in range(k_):
            acc -= v[:, j] * np.dot(u[:, j], u[:, k_])
        v[:, k_] = 2.0 * acc
    ut = np.ascontiguousarray(u.T).astype(ml_dtypes.bfloat16)          # [R, D]
    vn = np.ascontiguousarray(
        (-v).reshape(KT, P, R).transpose(1, 0, 2)
    ).astype(ml_dtypes.bfloat16)                                       # [P, KT, R]
    return ut, vn


def _make_in_maps(x, weight, hra_u):
    ut, vn = _host_prep(hra_u)
    xf = np.ascontiguousarray(x.reshape(TOK, D)).astype(ml_dtypes.bfloat16)
    wf = weight.astype(ml_dtypes.bfloat16)

    wts = []
    for b in range(O_GROUPS):
        ws = wf[b * O_S:(b + 1) * O_S]                     # [O_S, D]
        # wt[p, nb, k, t] = ws[nb*512 + t, k*128 + p]
        wt = np.ascontiguousarray(
            ws.reshape(NB, NBW, KT, P).transpose(3, 0, 2, 1)
        )
        wts.append(wt)

    in_maps = []
    for core in range(N_CORES):
        a, b = core // O_GROUPS, core % O_GROUPS
        in_maps.append({
            "x": np.ascontiguousarray(xf[a * TOK_S:(a + 1) * TOK_S]),
            "wt": wts[b],
            "ut": ut,
            "vn": vn,
        })
    return in_maps


def kernel(x, weight, hra_u):
    nc = _get_nc()
    in_maps = _make_in_maps(x, weight, hra_u)
    res = run_bass_kernel_spmd(nc, in_maps, core_ids=list(range(N_CORES))).results

    y = np.empty((TOK, O), dtype=np.float32)
    for core in range(N_CORES):
        a, b = core // O_GROUPS, core % O_GROUPS
        y[a * TOK_S:(a + 1) * TOK_S, b * O_S:(b + 1) * O_S] = res[core]["out"]
    return y.reshape(x.shape[0], x.shape[1], O)
